# revision 2
# baseline (speedup 1.0000x reference)
"""Trainium2 Bass kernel for nn_KANPointNet.

Structural insight: every KAN layer wires output channel j to input channel
j % Cin.  Walking the graph backward from the 40 output channels, only
channels 0..39 of layers 1, 2, 6, 7, 8, 9, 10 are live, and layer 6 reads
concat channels 0..39 which all fall in the `local` (layer-2) part — so the
entire max-pool branch (layers 3, 4, 5 + global pooling) is dead code.  The
network reduces to 40 independent per-channel chains of 7 cubic-spline
evaluations (+ ReLU between layers).

Numerical contract: the splines are DISCONTINUOUS at the knots and
intermediate values pass within 1 ulp of knot boundaries, so interval
selection must match the reference bit-for-bit.  XLA-CPU evaluates the
Horner polynomial with separately-rounded mult/add (verified: no FMA
contraction), which the per-op-rounded vector-engine ALUs reproduce
exactly.  Coefficient/knot selection uses one-hot masks (products with
exact 0.0/1.0), which is exact in any rounding mode.  Only the FINAL
layer's output (which feeds no further comparisons) is rounded to bf16
for the device->host transfer; that adds <=2^-8 relative error against
a 2e-2 gate.

Distribution: pure data-parallel over the B*N = 65536 points, 8192 per
core; no collectives (the max-pool that would have needed an
all-reduce-max is dead).  On-chip layout packs 3 point-groups x 40
channels onto 120 partitions; per-channel spline coefficients ride along
as per-partition scalars.

Dispatch: one jitted shard_map executable built once and cached.  The
zero "output" operands the bass_exec custom call wants are materialized
device-side (jnp.zeros inside the traced body) so nothing but the 786KB
of points crosses host->device per call; consts are cached on device
keyed by the weight bytes.
"""

import sys

import numpy as np

NCORES = 8
B, CIN, N = 8, 3, 8192
CH = 40                      # live channels
LAYERS = (1, 2, 6, 7, 8, 9, 10)
NL = len(LAYERS)
NI = 5                       # spline intervals (K-1)
GROUPS = 3
P = GROUPS * CH              # 120 partitions
PTS = B * N                  # 65536 total points
PTS_CORE = PTS // NCORES     # 8192 (== N, so core c handles batch b=c)
FREE = -(-PTS_CORE // GROUPS)  # 2731 (one padded point per core)
CPL = 24                     # const columns per layer: 20 coefs + 4 knots
NCHUNK = 3
PADCOLS = GROUPS * FREE      # 8193

_prog_cache = {}


def _build_program():
    """Build the Bass/Tile program (shape-only; all values arrive via DRAM)."""
    import concourse.bass as bass  # noqa: F401
    import concourse.mybir as mybir
    from concourse import bacc, tile

    f32 = mybir.dt.float32
    bf16 = mybir.dt.bfloat16
    Alu = mybir.AluOpType
    Act = mybir.ActivationFunctionType

    nc = bacc.Bacc(None, target_bir_lowering=False, debug=False,
                   enable_partition_id=False)
    v0_d = nc.declare_dram_parameter("v0", [CIN, PADCOLS], f32, isOutput=False)
    c_d = nc.declare_dram_parameter("consts", [P, NL * CPL], f32, isOutput=False)
    o_d = nc.declare_dram_parameter("out", [P, FREE], bf16, isOutput=True)

    fsz = [FREE // NCHUNK + (1 if i < FREE % NCHUNK else 0) for i in range(NCHUNK)]
    foff = [sum(fsz[:i]) for i in range(NCHUNK)]

    with tile.TileContext(nc) as tc:
        with (
            tc.tile_pool(name="cpool", bufs=1) as cpool,
            tc.tile_pool(name="vpool", bufs=2 * NCHUNK) as vpool,
            tc.tile_pool(name="pool", bufs=2) as pool,
        ):
            consts = cpool.tile([P, NL * CPL], f32, tag="consts")
            nc.sync.dma_start(consts[:], c_d[:])

            vcur = []
            for u in range(NCHUNK):
                vt = vpool.tile([P, fsz[u]], f32, tag="v")
                for g in range(GROUPS):
                    for c in range(CH):
                        p = g * CH + c
                        nc.sync.dma_start(
                            vt[p:p + 1, :],
                            v0_d[c % CIN:c % CIN + 1,
                                 g * FREE + foff[u]:g * FREE + foff[u] + fsz[u]])
                vcur.append(vt)

            for li in range(NL):
                cb = li * CPL

                def cc(m, k):
                    # coef k of interval m, per-partition scalar column
                    return consts[:, cb + m * 4 + k:cb + m * 4 + k + 1]

                def kt(j):
                    # knot t_{j+1} (j = 0..3)
                    return consts[:, cb + 20 + j:cb + 20 + j + 1]

                vnext = []
                for u in range(NCHUNK):
                    F = fsz[u]
                    V = vcur[u]
                    s = []
                    for j in range(4):
                        st = pool.tile([P, F], f32, tag=f"mask{j}")
                        nc.vector.tensor_scalar(
                            out=st[:], in0=V[:], scalar1=kt(j), scalar2=None,
                            op0=Alu.is_ge)
                        s.append(st)
                    ind0 = pool.tile([P, F], f32, tag="ind0")
                    nc.vector.tensor_scalar(
                        out=ind0[:], in0=s[0][:], scalar1=-1.0, scalar2=1.0,
                        op0=Alu.mult, op1=Alu.add)
                    ind = [ind0]
                    for j in range(3):
                        it = pool.tile([P, F], f32, tag=f"ind{j + 1}")
                        nc.gpsimd.tensor_tensor(
                            out=it[:], in0=s[j][:], in1=s[j + 1][:],
                            op=Alu.subtract)
                        ind.append(it)
                    ind.append(s[3])  # ind4 == s4

                    # knot select: T = sum_m ind_m * t_m   (t_0 == 0 skipped)
                    T = pool.tile([P, F], f32, tag="tsel")
                    nc.vector.tensor_scalar(
                        out=T[:], in0=ind[1][:], scalar1=kt(0), scalar2=None,
                        op0=Alu.mult)
                    for m in (2, 3, 4):
                        nc.vector.scalar_tensor_tensor(
                            out=T[:], in0=ind[m][:], scalar=kt(m - 1),
                            in1=T[:], op0=Alu.mult, op1=Alu.add)
                    dx = pool.tile([P, F], f32, tag="dx")
                    nc.vector.tensor_tensor(
                        out=dx[:], in0=V[:], in1=T[:], op=Alu.subtract)

                    # one-hot coefficient selection
                    X = []
                    for k in range(4):
                        eng = nc.vector
                        xt = pool.tile([P, F], f32, tag=f"x{k}")
                        eng.tensor_scalar(
                            out=xt[:], in0=ind[0][:], scalar1=cc(0, k),
                            scalar2=None, op0=Alu.mult)
                        for m in range(1, 5):
                            eng.scalar_tensor_tensor(
                                out=xt[:], in0=ind[m][:], scalar=cc(m, k),
                                in1=xt[:], op0=Alu.mult, op1=Alu.add)
                        X.append(xt)

                    # Horner, separately-rounded to match the reference:
                    # y = ((c0*dx + c1)*dx + c2)*dx + c3
                    h = pool.tile([P, F], f32, tag="h")
                    nc.vector.tensor_tensor(out=h[:], in0=X[0][:], in1=dx[:], op=Alu.mult)
                    nc.vector.tensor_tensor(out=h[:], in0=h[:], in1=X[1][:], op=Alu.add)
                    nc.vector.tensor_tensor(out=h[:], in0=h[:], in1=dx[:], op=Alu.mult)
                    nc.vector.tensor_tensor(out=h[:], in0=h[:], in1=X[2][:], op=Alu.add)
                    nc.vector.tensor_tensor(out=h[:], in0=h[:], in1=dx[:], op=Alu.mult)

                    if li < NL - 1:
                        y = pool.tile([P, F], f32, tag="y")
                        nc.vector.tensor_tensor(out=y[:], in0=h[:], in1=X[3][:], op=Alu.add)
                        vn = vpool.tile([P, F], f32, tag="v")
                        nc.scalar.activation(out=vn[:], in_=y[:], func=Act.Relu)
                        vnext.append(vn)
                    else:
                        # final layer: round to bf16 for the D2H transfer
                        yb = pool.tile([P, F], bf16, tag="yb")
                        nc.vector.tensor_tensor(out=yb[:], in0=h[:], in1=X[3][:], op=Alu.add)
                        nc.sync.dma_start(
                            o_d[:, foff[u]:foff[u] + fsz[u]], yb[:])
                vcur = vnext

    nc.compile()
    from concourse.bass_interp import get_hw_module
    nc.m = get_hw_module(nc.m)
    return nc


def _get_program():
    if "nc" not in _prog_cache:
        _prog_cache["nc"] = _build_program()
    return _prog_cache["nc"]


def _pack_points(x):
    """x [B,3,N] f32 -> v0 global [NCORES*CIN, PADCOLS] (core c rows 3c:3c+3)."""
    v0 = np.empty((NCORES * CIN, PADCOLS), dtype=np.float32)
    v0[:, :PTS_CORE] = x.reshape(NCORES * CIN, PTS_CORE)
    v0[:, PTS_CORE:] = 0.5
    return v0


def _pack_consts(inputs):
    """Per-core consts [P, NL*CPL]; identical for every core."""
    consts = np.zeros((P, NL * CPL), dtype=np.float32)
    for li, ref_l in enumerate(LAYERS):
        kn = np.asarray(inputs[f"knots{ref_l}"], dtype=np.float32)[:CH]
        cf = np.asarray(inputs[f"coefs{ref_l}"], dtype=np.float32)[:CH]
        assert np.all(kn[:, 0] == 0.0), "kernel assumes knots start at 0"
        assert np.all(kn == kn[0][None, :]), "kernel assumes shared knots per layer"
        base = li * CPL
        for m in range(NI):
            for k in range(4):
                consts[:, base + m * 4 + k] = np.tile(cf[:, k, m], GROUPS)
        for j in range(4):
            consts[:, base + 20 + j] = kn[0, j + 1]
    return consts


def _unpack_output(out_concat):
    """[NCORES*P, FREE] bf16 -> [B, CH, N] f32."""
    o = np.asarray(out_concat).astype(np.float32)
    o = o.reshape(NCORES, GROUPS, CH, FREE).transpose(0, 2, 1, 3)
    return np.ascontiguousarray(o.reshape(B, CH, PADCOLS)[:, :, :PTS_CORE])


def _build_fast_callable(nc):
    """One-time jitted shard_map wrapper around the compiled BIR kernel.

    Mirrors concourse.bass2jax.run_bass_via_pjrt but (a) is traced/compiled
    exactly once, (b) materializes the donated zero output operands on
    device inside the traced body (no per-call H2D of zeros), and (c) uses
    the effect-free C++ fast dispatch path when available.
    """
    import jax
    import jax.numpy as jnp
    from jax.sharding import Mesh, NamedSharding, PartitionSpec
    from jax.experimental.shard_map import shard_map
    import concourse.mybir as mybir
    from concourse import bass2jax

    bass2jax.install_neuronx_cc_hook()
    hw = nc.m  # already the hw module (set in _build_program)

    in_names, out_names, out_avals = [], [], []
    for alloc in hw.functions[0].allocations:
        if not isinstance(alloc, mybir.MemoryLocationSet):
            continue
        name = alloc.memorylocations[0].name
        if alloc.kind == "ExternalInput":
            in_names.append(name)
        elif alloc.kind == "ExternalOutput":
            out_names.append(name)
            out_avals.append(jax.core.ShapedArray(
                tuple(alloc.tensor_shape), mybir.dt.np(alloc.dtype)))
    assert sorted(in_names) == ["consts", "v0"], in_names
    assert out_names == ["out"], out_names
    in_names = ["v0", "consts"]  # fixed order for the bind below

    def _body(v0, consts):
        zeros = [jnp.zeros(a.shape, a.dtype) for a in out_avals]
        outs = bass2jax._bass_exec_p.bind(
            v0, consts, *zeros,
            out_avals=tuple(out_avals),
            in_names=tuple(in_names) + tuple(out_names),
            out_names=tuple(out_names),
            lowering_input_output_aliases=(),
            sim_require_finite=True,
            sim_require_nnan=True,
            nc=nc,
        )
        return outs[0]

    devices = jax.devices()[:NCORES]
    assert len(devices) >= NCORES, f"need {NCORES} devices, have {len(devices)}"
    mesh = Mesh(np.asarray(devices), ("core",))
    spec = PartitionSpec("core")
    sharding = NamedSharding(mesh, spec)
    fn = shard_map(_body, mesh=mesh, in_specs=(spec, spec), out_specs=spec,
                   check_rep=False)

    v0_sds = jax.ShapeDtypeStruct(
        (NCORES * CIN, PADCOLS), np.float32, sharding=sharding)
    c_sds = jax.ShapeDtypeStruct(
        (NCORES * P, NL * CPL), np.float32, sharding=sharding)
    try:
        compiled = bass2jax.fast_dispatch_compile(
            lambda: jax.jit(fn).lower(v0_sds, c_sds).compile())
    except Exception as e:  # pragma: no cover - fall back to normal dispatch
        print(f"kernel.py: fast_dispatch_compile failed ({e!r}); "
              "using plain jit", file=sys.stderr)
        compiled = jax.jit(fn)

    _prog_cache["sharding"] = sharding
    return compiled


def _get_consts_dev(inputs):
    """Device-resident per-core-replicated consts, cached on weight bytes."""
    import jax
    key = b"".join(
        np.asarray(inputs[f"{nm}{li}"], dtype=np.float32).tobytes()
        for li in LAYERS for nm in ("knots", "coefs"))
    import hashlib
    digest = hashlib.blake2b(key, digest_size=16).digest()
    if _prog_cache.get("consts_key") != digest:
        consts = _pack_consts(inputs)
        consts_g = np.tile(consts, (NCORES, 1))
        _prog_cache["consts_dev"] = jax.device_put(
            consts_g, _prog_cache["sharding"])
        _prog_cache["consts_key"] = digest
    return _prog_cache["consts_dev"]


def run(inputs, trace=False):
    """Run on the 8 NeuronCores; returns (output, BassKernelResults)."""
    from concourse.bass_utils import BassKernelResults

    nc = _get_program()
    if "fast" not in _prog_cache:
        _prog_cache["fast"] = _build_fast_callable(nc)

    x = np.ascontiguousarray(np.asarray(inputs["x"], dtype=np.float32))
    assert x.shape == (B, CIN, N), x.shape

    if trace:
        return _run_traced(nc, x, inputs)

    v0 = _pack_points(x)
    consts_dev = _get_consts_dev(inputs)
    try:
        out = _prog_cache["fast"](v0, consts_dev)
        res = BassKernelResults(
            results=None, instructions_and_trace=None,
            profile_json=None, exec_time_ns=None)
        return _unpack_output(out), res
    except Exception as e:
        print(f"kernel.py: fast path failed ({e!r}); falling back to "
              "run_bass_kernel_spmd", file=sys.stderr)
        return _run_traced(nc, x, inputs, trace=False)


def _run_traced(nc, x, inputs, trace=True):
    """Slow path through run_bass_kernel_spmd (used for NTFF profiling)."""
    from concourse.bass_utils import run_bass_kernel_spmd

    v0 = _pack_points(x)
    consts = _pack_consts(inputs)
    in_maps = [
        {"v0": v0[c * CIN:(c + 1) * CIN], "consts": consts}
        for c in range(NCORES)]
    res = run_bass_kernel_spmd(
        nc, in_maps, core_ids=list(range(NCORES)), trace=trace)
    out = np.concatenate([r["out"] for r in res.results], axis=0)
    return _unpack_output(out), res


def kernel(**inputs) -> np.ndarray:
    out, _ = run(inputs, trace=False)
    return out


# revision 4
# speedup vs baseline: 2.8117x; 2.8117x over previous
"""Trainium2 Bass kernel for nn_KANPointNet.

Structural insight: every KAN layer wires output channel j to input channel
j % Cin.  Walking the graph backward from the 40 output channels, only
channels 0..39 of layers 1, 2, 6, 7, 8, 9, 10 are live, and layer 6 reads
concat channels 0..39 which all fall in the `local` (layer-2) part — so the
entire max-pool branch (layers 3, 4, 5 + global pooling) is dead code.  The
network reduces to 40 independent per-channel chains of 7 cubic-spline
evaluations (+ ReLU between layers).

Numerical contract: the splines are DISCONTINUOUS at the knots and
intermediate values pass within 1 ulp of knot boundaries, so interval
selection must match the reference bit-for-bit.  XLA-CPU evaluates the
Horner polynomial with separately-rounded mult/add (verified: no FMA
contraction), which the per-op-rounded vector-engine ALUs reproduce
exactly.  Coefficient/knot selection uses one-hot masks (products with
exact 0.0/1.0), which is exact in any rounding mode.  Only the FINAL
layer's output (which feeds no further comparisons) is rounded to bf16
for the device->host transfer; that adds <=2^-8 relative error against
a 2e-2 gate.

Distribution: pure data-parallel over the B*N = 65536 points, 8192 per
core; no collectives (the max-pool that would have needed an
all-reduce-max is dead).  On-chip layout packs 3 point-groups x 40
channels onto 120 partitions; per-channel spline coefficients ride along
as per-partition scalars.

Dispatch: one jitted shard_map executable built once and cached.  The
zero "output" operands the bass_exec custom call wants are materialized
device-side (jnp.zeros inside the traced body) so nothing but the 786KB
of points crosses host->device per call; consts are cached on device
keyed by the weight bytes.
"""

import sys

import numpy as np

NCORES = 8
B, CIN, N = 8, 3, 8192
CH = 40                      # live channels
LAYERS = (1, 2, 6, 7, 8, 9, 10)
NL = len(LAYERS)
NI = 5                       # spline intervals (K-1)
GROUPS = 3
P = GROUPS * CH              # 120 partitions
PTS = B * N                  # 65536 total points
PTS_CORE = PTS // NCORES     # 8192 (== N, so core c handles batch b=c)
FREE = -(-PTS_CORE // GROUPS)  # 2731 (one padded point per core)
CPL = 24                     # const columns per layer: 20 coefs + 4 knots
NCHUNK = 3
PADCOLS = GROUPS * FREE      # 8193

_prog_cache = {}


def _build_program():
    """Build the Bass/Tile program (shape-only; all values arrive via DRAM)."""
    import concourse.bass as bass  # noqa: F401
    import concourse.mybir as mybir
    from concourse import bacc, tile

    f32 = mybir.dt.float32
    bf16 = mybir.dt.bfloat16
    Alu = mybir.AluOpType
    Act = mybir.ActivationFunctionType

    nc = bacc.Bacc(None, target_bir_lowering=False, debug=False,
                   enable_partition_id=False)
    v0_d = nc.declare_dram_parameter("v0", [CIN, PADCOLS], f32, isOutput=False)
    c_d = nc.declare_dram_parameter("consts", [P, NL * CPL], f32, isOutput=False)
    o_d = nc.declare_dram_parameter("out", [P, FREE], bf16, isOutput=True)

    fsz = [FREE // NCHUNK + (1 if i < FREE % NCHUNK else 0) for i in range(NCHUNK)]
    foff = [sum(fsz[:i]) for i in range(NCHUNK)]

    with tile.TileContext(nc) as tc:
        with (
            tc.tile_pool(name="cpool", bufs=1) as cpool,
            tc.tile_pool(name="vpool", bufs=2 * NCHUNK) as vpool,
            tc.tile_pool(name="pool", bufs=2) as pool,
        ):
            consts = cpool.tile([P, NL * CPL], f32, tag="consts")
            nc.sync.dma_start(consts[:], c_d[:])

            vcur = []
            for u in range(NCHUNK):
                vt = vpool.tile([P, fsz[u]], f32, tag="v")
                for g in range(GROUPS):
                    for c in range(CH):
                        p = g * CH + c
                        nc.sync.dma_start(
                            vt[p:p + 1, :],
                            v0_d[c % CIN:c % CIN + 1,
                                 g * FREE + foff[u]:g * FREE + foff[u] + fsz[u]])
                vcur.append(vt)

            for li in range(NL):
                cb = li * CPL

                def cc(m, k):
                    # coef k of interval m, per-partition scalar column
                    return consts[:, cb + m * 4 + k:cb + m * 4 + k + 1]

                def kt(j):
                    # knot t_{j+1} (j = 0..3)
                    return consts[:, cb + 20 + j:cb + 20 + j + 1]

                vnext = []
                for u in range(NCHUNK):
                    F = fsz[u]
                    V = vcur[u]
                    s = []
                    for j in range(4):
                        st = pool.tile([P, F], f32, tag=f"mask{j}")
                        nc.vector.tensor_scalar(
                            out=st[:], in0=V[:], scalar1=kt(j), scalar2=None,
                            op0=Alu.is_ge)
                        s.append(st)
                    ind0 = pool.tile([P, F], f32, tag="ind0")
                    nc.vector.tensor_scalar(
                        out=ind0[:], in0=s[0][:], scalar1=-1.0, scalar2=1.0,
                        op0=Alu.mult, op1=Alu.add)
                    ind = [ind0]
                    for j in range(3):
                        it = pool.tile([P, F], f32, tag=f"ind{j + 1}")
                        nc.gpsimd.tensor_tensor(
                            out=it[:], in0=s[j][:], in1=s[j + 1][:],
                            op=Alu.subtract)
                        ind.append(it)
                    ind.append(s[3])  # ind4 == s4

                    # knot select: T = sum_m ind_m * t_m   (t_0 == 0 skipped)
                    T = pool.tile([P, F], f32, tag="tsel")
                    nc.vector.tensor_scalar(
                        out=T[:], in0=ind[1][:], scalar1=kt(0), scalar2=None,
                        op0=Alu.mult)
                    for m in (2, 3, 4):
                        nc.vector.scalar_tensor_tensor(
                            out=T[:], in0=ind[m][:], scalar=kt(m - 1),
                            in1=T[:], op0=Alu.mult, op1=Alu.add)
                    dx = pool.tile([P, F], f32, tag="dx")
                    nc.vector.tensor_tensor(
                        out=dx[:], in0=V[:], in1=T[:], op=Alu.subtract)

                    # one-hot coefficient selection
                    X = []
                    for k in range(4):
                        eng = nc.vector
                        xt = pool.tile([P, F], f32, tag=f"x{k}")
                        eng.tensor_scalar(
                            out=xt[:], in0=ind[0][:], scalar1=cc(0, k),
                            scalar2=None, op0=Alu.mult)
                        for m in range(1, 5):
                            eng.scalar_tensor_tensor(
                                out=xt[:], in0=ind[m][:], scalar=cc(m, k),
                                in1=xt[:], op0=Alu.mult, op1=Alu.add)
                        X.append(xt)

                    # Horner, separately-rounded to match the reference:
                    # y = ((c0*dx + c1)*dx + c2)*dx + c3
                    h = pool.tile([P, F], f32, tag="h")
                    nc.vector.tensor_tensor(out=h[:], in0=X[0][:], in1=dx[:], op=Alu.mult)
                    nc.vector.tensor_tensor(out=h[:], in0=h[:], in1=X[1][:], op=Alu.add)
                    nc.vector.tensor_tensor(out=h[:], in0=h[:], in1=dx[:], op=Alu.mult)
                    nc.vector.tensor_tensor(out=h[:], in0=h[:], in1=X[2][:], op=Alu.add)
                    nc.vector.tensor_tensor(out=h[:], in0=h[:], in1=dx[:], op=Alu.mult)

                    if li < NL - 1:
                        y = pool.tile([P, F], f32, tag="y")
                        nc.vector.tensor_tensor(out=y[:], in0=h[:], in1=X[3][:], op=Alu.add)
                        vn = vpool.tile([P, F], f32, tag="v")
                        nc.scalar.activation(out=vn[:], in_=y[:], func=Act.Relu)
                        vnext.append(vn)
                    else:
                        # final layer: round to bf16 for the D2H transfer
                        yb = pool.tile([P, F], bf16, tag="yb")
                        nc.vector.tensor_tensor(out=yb[:], in0=h[:], in1=X[3][:], op=Alu.add)
                        nc.sync.dma_start(
                            o_d[:, foff[u]:foff[u] + fsz[u]], yb[:])
                vcur = vnext

    nc.compile()
    from concourse.bass_interp import get_hw_module
    nc.m = get_hw_module(nc.m)
    return nc


def _get_program():
    if "nc" not in _prog_cache:
        _prog_cache["nc"] = _build_program()
    return _prog_cache["nc"]


def _pack_points(x):
    """x [B,3,N] f32 -> v0 global [NCORES*CIN, PADCOLS] (core c rows 3c:3c+3)."""
    v0 = np.empty((NCORES * CIN, PADCOLS), dtype=np.float32)
    v0[:, :PTS_CORE] = x.reshape(NCORES * CIN, PTS_CORE)
    v0[:, PTS_CORE:] = 0.5
    return v0


def _pack_consts(inputs):
    """Per-core consts [P, NL*CPL]; identical for every core."""
    consts = np.zeros((P, NL * CPL), dtype=np.float32)
    for li, ref_l in enumerate(LAYERS):
        kn = np.asarray(inputs[f"knots{ref_l}"], dtype=np.float32)[:CH]
        cf = np.asarray(inputs[f"coefs{ref_l}"], dtype=np.float32)[:CH]
        assert np.all(kn[:, 0] == 0.0), "kernel assumes knots start at 0"
        assert np.all(kn == kn[0][None, :]), "kernel assumes shared knots per layer"
        base = li * CPL
        for m in range(NI):
            for k in range(4):
                consts[:, base + m * 4 + k] = np.tile(cf[:, k, m], GROUPS)
        for j in range(4):
            consts[:, base + 20 + j] = kn[0, j + 1]
    return consts


def _unpack_output(out_concat):
    """[NCORES*P, FREE] bf16 -> [B, CH, N] f32."""
    o = np.asarray(out_concat).astype(np.float32)
    o = o.reshape(NCORES, GROUPS, CH, FREE).transpose(0, 2, 1, 3)
    return np.ascontiguousarray(o.reshape(B, CH, PADCOLS)[:, :, :PTS_CORE])


def _build_fast_callable(nc):
    """One-time jitted shard_map wrapper around the compiled BIR kernel.

    Mirrors concourse.bass2jax.run_bass_via_pjrt but (a) is traced/compiled
    exactly once, (b) materializes the donated zero output operands on
    device inside the traced body (no per-call H2D of zeros), and (c) uses
    the effect-free C++ fast dispatch path when available.
    """
    import jax
    import jax.numpy as jnp
    from jax.sharding import Mesh, NamedSharding, PartitionSpec
    from jax.experimental.shard_map import shard_map
    import concourse.mybir as mybir
    from concourse import bass2jax

    bass2jax.install_neuronx_cc_hook()
    hw = nc.m  # already the hw module (set in _build_program)

    in_names, out_names, out_avals = [], [], []
    for alloc in hw.functions[0].allocations:
        if not isinstance(alloc, mybir.MemoryLocationSet):
            continue
        name = alloc.memorylocations[0].name
        if alloc.kind == "ExternalInput":
            in_names.append(name)
        elif alloc.kind == "ExternalOutput":
            out_names.append(name)
            out_avals.append(jax.core.ShapedArray(
                tuple(alloc.tensor_shape), mybir.dt.np(alloc.dtype)))
    assert sorted(in_names) == ["consts", "v0"], in_names
    assert out_names == ["out"], out_names
    in_names = ["v0", "consts"]  # fixed order for the bind below

    def _body(v0, consts, outbuf):
        outs = bass2jax._bass_exec_p.bind(
            v0, consts, outbuf,
            out_avals=tuple(out_avals),
            in_names=tuple(in_names) + tuple(out_names),
            out_names=tuple(out_names),
            lowering_input_output_aliases=(),
            sim_require_finite=True,
            sim_require_nnan=True,
            nc=nc,
        )
        return outs[0]

    devices = jax.devices()[:NCORES]
    assert len(devices) >= NCORES, f"need {NCORES} devices, have {len(devices)}"
    mesh = Mesh(np.asarray(devices), ("core",))
    spec = PartitionSpec("core")
    sharding = NamedSharding(mesh, spec)
    fn = shard_map(_body, mesh=mesh, in_specs=(spec, spec, spec),
                   out_specs=spec, check_rep=False)

    # device-resident "output" operand for the bass_exec custom call;
    # transferred once and reused every call (the kernel writes every
    # element of out, so its contents never matter).
    outbuf_np = np.zeros(
        (NCORES * out_avals[0].shape[0],) + tuple(out_avals[0].shape[1:]),
        out_avals[0].dtype)
    _prog_cache["outbuf_dev"] = jax.device_put(outbuf_np, sharding)

    v0_sds = jax.ShapeDtypeStruct(
        (NCORES * CIN, PADCOLS), np.float32, sharding=sharding)
    c_sds = jax.ShapeDtypeStruct(
        (NCORES * P, NL * CPL), np.float32, sharding=sharding)
    o_sds = jax.ShapeDtypeStruct(
        outbuf_np.shape, outbuf_np.dtype, sharding=sharding)
    try:
        compiled = bass2jax.fast_dispatch_compile(
            lambda: jax.jit(fn).lower(v0_sds, c_sds, o_sds).compile())
    except Exception as e:  # pragma: no cover - fall back to normal dispatch
        print(f"kernel.py: fast_dispatch_compile failed ({e!r}); "
              "using plain jit", file=sys.stderr)
        compiled = jax.jit(fn)

    _prog_cache["sharding"] = sharding
    return compiled


def _get_consts_dev(inputs):
    """Device-resident per-core-replicated consts, cached on weight bytes."""
    import jax
    key = b"".join(
        np.asarray(inputs[f"{nm}{li}"], dtype=np.float32).tobytes()
        for li in LAYERS for nm in ("knots", "coefs"))
    import hashlib
    digest = hashlib.blake2b(key, digest_size=16).digest()
    if _prog_cache.get("consts_key") != digest:
        consts = _pack_consts(inputs)
        consts_g = np.tile(consts, (NCORES, 1))
        _prog_cache["consts_dev"] = jax.device_put(
            consts_g, _prog_cache["sharding"])
        _prog_cache["consts_key"] = digest
    return _prog_cache["consts_dev"]


def run(inputs, trace=False):
    """Run on the 8 NeuronCores; returns (output, BassKernelResults)."""
    from concourse.bass_utils import BassKernelResults

    nc = _get_program()
    if "fast" not in _prog_cache:
        _prog_cache["fast"] = _build_fast_callable(nc)

    x = np.ascontiguousarray(np.asarray(inputs["x"], dtype=np.float32))
    assert x.shape == (B, CIN, N), x.shape

    if trace:
        return _run_traced(nc, x, inputs)

    v0 = _pack_points(x)
    consts_dev = _get_consts_dev(inputs)
    try:
        out = _prog_cache["fast"](v0, consts_dev, _prog_cache["outbuf_dev"])
        res = BassKernelResults(
            results=None, instructions_and_trace=None,
            profile_json=None, exec_time_ns=None)
        return _unpack_output(out), res
    except Exception as e:
        print(f"kernel.py: fast path failed ({e!r}); falling back to "
              "run_bass_kernel_spmd", file=sys.stderr)
        return _run_traced(nc, x, inputs, trace=False)


def _run_traced(nc, x, inputs, trace=True):
    """Slow path through run_bass_kernel_spmd (used for NTFF profiling)."""
    from concourse.bass_utils import run_bass_kernel_spmd

    v0 = _pack_points(x)
    consts = _pack_consts(inputs)
    in_maps = [
        {"v0": v0[c * CIN:(c + 1) * CIN], "consts": consts}
        for c in range(NCORES)]
    res = run_bass_kernel_spmd(
        nc, in_maps, core_ids=list(range(NCORES)), trace=trace)
    out = np.concatenate([r["out"] for r in res.results], axis=0)
    return _unpack_output(out), res


def kernel(**inputs) -> np.ndarray:
    out, _ = run(inputs, trace=False)
    return out
